# revision 1
# baseline (speedup 1.0000x reference)
"""Trainium2 Bass kernel for a PointNet++-style set-abstraction layer.

Per batch element: farthest-point sampling (1024 sequential steps), radius
ball-query grouping, pointwise MLP, 1x1 conv + global BatchNorm + ReLU,
neighborhood max-pool.  Data-parallel over batch: one batch element per
NeuronCore, with a single AllReduce for the BatchNorm statistics.

Key restructurings vs. the reference (validated to rel-err ~1e-6 on CPU):
  - FPS extracts the new centroid's coordinates via equality masks and
    cross-partition reductions (no integer argmax index needed).
  - Ball-query "first 32 in-radius indices" built with a prefix-scan rank +
    GPSIMD local_scatter (slot = rank-1 for hits with rank<=32).
  - q = (points @ W1 + b1) @ Wc + bc is computed once per point (instead of
    per gathered duplicate); the neighborhood max-pool is a DMA row gather of
    q followed by a free-axis max reduce.  max-pool commutes with the
    monotone BN+ReLU (gamma > 0), so BN is applied after pooling.
  - BN mean/var come from multiplicity-weighted sums: sum_i w_i q_i (+ a
    padding-duplicate correction term), AllReduced across cores.
"""

import os
import sys

if "/opt/trn_rl_repo" not in sys.path:
    sys.path.insert(0, "/opt/trn_rl_repo")

import numpy as np

B = 8
N = 4096
S = 1024
NS = 32
CIN = 64
CMLP = 128
COUT = 256
RADIUS2 = float(np.float32(np.float64(0.15) * np.float64(0.15)))
BN_EPS = 1e-5


def build_nc(n=N, s=S, ns=NS, cin=CIN, cmlp=CMLP, cout=COUT, n_cores=B,
             batch_total=None, stop_after=None):
    """Emit the Bass module (identical program on every core)."""
    from contextlib import ExitStack

    import concourse.bass as bass
    import concourse.tile as tile
    from concourse import bacc, bass_isa, mybir

    f32 = mybir.dt.float32
    bf16 = mybir.dt.bfloat16
    i16 = mybir.dt.int16
    Alu = mybir.AluOpType
    Act = mybir.ActivationFunctionType
    Red = bass_isa.ReduceOp

    FF = n // 128          # free elems per coordinate plane in FPS layout
    SC = s // 128          # center chunks
    PC = n // 128          # point chunks (q rows)
    NFB = n // 512         # 512-wide free blocks of n
    NSCATTER = 4           # local_scatter sub-calls per center chunk
    NSUB = n // NSCATTER
    if batch_total is None:
        batch_total = n_cores
    CNT = float(batch_total * s * ns)

    class _StopEmit(Exception):
        pass

    nc = bacc.Bacc("TRN2", target_bir_lowering=False, debug=False,
                   num_devices=n_cores)

    xyzT_d = nc.dram_tensor("xyzT", [3, n], f32, kind="ExternalInput")
    pointsT_d = nc.dram_tensor("pointsT", [cin, n], f32, kind="ExternalInput")
    W1_d = nc.dram_tensor("W1", [cin, cmlp], f32, kind="ExternalInput")
    b1_d = nc.dram_tensor("b1", [1, cmlp], f32, kind="ExternalInput")
    Wc_d = nc.dram_tensor("Wc", [cmlp, cout], f32, kind="ExternalInput")
    bc_d = nc.dram_tensor("bc", [1, cout], f32, kind="ExternalInput")
    gamma_d = nc.dram_tensor("gamma", [1, cout], f32, kind="ExternalInput")
    beta_d = nc.dram_tensor("beta", [1, cout], f32, kind="ExternalInput")
    out_d = nc.dram_tensor("out", [s, cout], f32, kind="ExternalOutput")

    qdram = nc.dram_tensor("qdram", [n, cout], f32)
    gidxdram = nc.dram_tensor("gidxdram", [s, ns], i16)
    wdram = nc.dram_tensor("wdram", [1, n], f32)
    nxdram = nc.dram_tensor("nxdram", [1, 3 * s], f32)
    ccin_d = nc.dram_tensor("ccin", [1, 2 * cout], f32)
    ccout_d = nc.dram_tensor("ccout", [1, 2 * cout], f32)

    with tile.TileContext(nc) as tc, ExitStack() as ctx:
      try:
        const = ctx.enter_context(tc.tile_pool(name="const", bufs=1))

        # ---- constant / input loads -----------------------------------
        W1_sb = const.tile([cin, cmlp], f32)
        nc.sync.dma_start(W1_sb[:], W1_d.ap())
        Wc_sb = const.tile([cmlp, cout], f32)
        nc.sync.dma_start(Wc_sb[:], Wc_d.ap())
        b1row = const.tile([1, cmlp], f32)
        nc.sync.dma_start(b1row[:], b1_d.ap())
        bcrow = const.tile([1, cout], f32)
        nc.sync.dma_start(bcrow[:], bc_d.ap())
        gammarow = const.tile([1, cout], f32)
        nc.sync.dma_start(gammarow[:], gamma_d.ap())
        betarow = const.tile([1, cout], f32)
        nc.sync.dma_start(betarow[:], beta_d.ap())
        ones512 = const.tile([1, 512], f32)
        nc.vector.memset(ones512[:], 1.0)
        onesK1 = const.tile([1, 128], f32)
        nc.vector.memset(onesK1[:], 1.0)
        ones128bf = const.tile([128, 1], bf16)
        nc.vector.memset(ones128bf[:], 1.0)
        zeros1_bf = const.tile([128, 1], bf16)
        nc.vector.memset(zeros1_bf[:], 0.0)
        # X3[p, j*FF + f] = xyz[p*FF + f, j]
        X3 = const.tile([128, 3 * FF], f32)
        for j in range(3):
            src = bass.AP(xyzT_d, j * n, [[FF, 128], [1, FF]])
            nc.sync.dma_start(X3[:, j * FF:(j + 1) * FF], src)
        X3v = X3[:, :].rearrange("p (j f) -> p j f", j=3)

        negc = const.tile([128, 3 * SC], f32)       # -new_xyz, per-chunk cols
        nxrow = const.tile([1, 3 * s], f32)         # new_xyz as partition-0 row
        pooled_all = const.tile([128, SC * cout], f32)
        padcnt_all = const.tile([128, SC], f32)
        wT = const.tile([128, PC], f32)

        # ---- phase A: featsT = W1^T @ pointsT + b1; q rows -> qdram ----
        with tc.tile_pool(name="psA", bufs=2, space="PSUM") as psA, \
             tc.tile_pool(name="qtmp", bufs=3) as qtmp, \
             tc.tile_pool(name="phA", bufs=1) as phA:
            pointsT_sb = phA.tile([cin, n], f32)
            nc.sync.dma_start(pointsT_sb[:], pointsT_d.ap())
            featsT_sb = phA.tile([cmlp, n], f32)
            for blk in range(NFB):
                ps = psA.tile([128, 512], f32, tag="fts")
                nc.tensor.matmul(ps[:], lhsT=b1row[:], rhs=ones512[:],
                                 start=True, stop=False)
                nc.tensor.matmul(ps[:], lhsT=W1_sb[:],
                                 rhs=pointsT_sb[:, blk * 512:(blk + 1) * 512],
                                 start=False, stop=True)
                nc.scalar.copy(featsT_sb[:, blk * 512:(blk + 1) * 512], ps[:])
            for c in range(PC):
                qp = psA.tile([128, cout], f32, tag="qp")
                nc.tensor.matmul(qp[:], lhsT=onesK1[:], rhs=bcrow[:],
                                 start=True, stop=False)
                nc.tensor.matmul(qp[:], lhsT=featsT_sb[:, c * 128:(c + 1) * 128],
                                 rhs=Wc_sb[:], start=False, stop=True)
                qrow = qtmp.tile([128, cout], f32, tag="qrow")
                nc.scalar.copy(qrow[:], qp[:])
                nc.sync.dma_start(qdram.ap()[c * 128:(c + 1) * 128, :], qrow[:])

        if stop_after == "A":
            dbg = const.tile([128, cout], f32)
            nc.sync.dma_start(dbg[:], qdram.ap()[0:128, :])
            nc.sync.dma_start(out_d.ap()[0:128, :], dbg[:])
            raise _StopEmit()

        # ---- phase B: farthest point sampling -------------------------
        # Cross-partition argmax/extraction via PE (transpose + all-ones
        # matmul that column-sums AND broadcasts to every partition).
        from concourse.masks import make_identity

        ident = const.tile([128, 128], f32)
        make_identity(nc, ident[:])
        onesSQ = const.tile([128, 128], f32)
        nc.vector.memset(onesSQ[:], 1.0)
        ones128f = const.tile([128, 1], f32)
        nc.vector.memset(ones128f[:], 1.0)

        with tc.tile_pool(name="fps", bufs=2) as fp, \
             tc.tile_pool(name="fpp", bufs=2, space="PSUM") as fpp, \
             tc.tile_pool(name="fps1", bufs=1) as fp1:
            dmin = fp1.tile([128, FF], f32)
            nc.vector.memset(dmin[:], 1e10)

            def extract_tail(mask_ap, k):
                """masked point's coords -> csumB [128,3] PSUM (all parts
                equal); coords also appended to nxrow (partition-0 row)."""
                mx3 = fp.tile([128, 3 * FF], f32, tag="mx3")
                nc.vector.tensor_tensor(
                    mx3[:, :].rearrange("p (j f) -> p j f", j=3), X3v,
                    mask_ap, Alu.mult)
                m3 = fp.tile([128, 3], f32, tag="m3")
                nc.vector.tensor_reduce(
                    m3[:], mx3[:, :].rearrange("p (j f) -> p j f", j=3),
                    axis=mybir.AxisListType.X, op=Alu.add)
                csumB = fpp.tile([128, 3], f32, tag="csumB")
                nc.tensor.matmul(csumB[:], lhsT=onesSQ[:], rhs=m3[:],
                                 start=True, stop=True)
                nc.scalar.copy(nxrow[:, 3 * k:3 * k + 3], csumB[0:1, :])
                return csumB

            msk0 = fp1.tile([128, FF], f32)
            nc.vector.memset(msk0[:], 0.0)
            nc.vector.memset(msk0[0:1, 0:1], 1.0)
            csumB = extract_tail(
                msk0[:, :].unsqueeze(1).broadcast_to([128, 3, FF]), 0)

            for k in range(1, s):
                diff = fp.tile([128, 3 * FF], f32, tag="diff")
                nc.vector.tensor_tensor(
                    diff[:, :].rearrange("p (j f) -> p j f", j=3), X3v,
                    csumB[:, :].unsqueeze(2).broadcast_to([128, 3, FF]),
                    Alu.subtract)
                sq = fp.tile([128, 3 * FF], f32, tag="sq")
                nc.vector.tensor_tensor(sq[:], diff[:], diff[:], Alu.mult)
                d = fp.tile([128, FF], f32, tag="d")
                nc.vector.tensor_reduce(
                    d[:], sq[:, :].rearrange("p (j f) -> p f j", j=3),
                    axis=mybir.AxisListType.X, op=Alu.add)
                dmin2 = fp.tile([128, FF], f32, tag="dmin2")
                nc.vector.tensor_tensor(dmin2[:], d[:], dmin[:], Alu.min)
                dmin = dmin2
                permax = fp.tile([128, 1], f32, tag="permax")
                nc.vector.tensor_reduce(permax[:], dmin2[:],
                                        axis=mybir.AxisListType.X, op=Alu.max)
                pmT = fpp.tile([1, 128], f32, tag="pmT")
                nc.tensor.transpose(pmT[:], permax[:], ident[:])
                gmax = fp.tile([1, 1], f32, tag="gmax")
                nc.vector.tensor_reduce(gmax[:], pmT[:],
                                        axis=mybir.AxisListType.X, op=Alu.max)
                gmaxB = fpp.tile([128, 1], f32, tag="gmaxB")
                nc.tensor.matmul(gmaxB[:], lhsT=onesK1[:], rhs=gmax[:],
                                 start=True, stop=True)
                gmask = fp.tile([128, FF], f32, tag="gmask")
                nc.vector.tensor_scalar(gmask[:], dmin2[:], gmaxB[:, 0:1],
                                        None, op0=Alu.is_equal)
                csumB = extract_tail(
                    gmask[:, :].unsqueeze(1).broadcast_to([128, 3, FF]), k)

        if stop_after == "B":
            nc.sync.dma_start(out_d.ap()[0:1, 0:cout], nxrow[:, 0:cout])
            raise _StopEmit()

        # negc[p, 3t+j] = -new_xyz[128t+p, j] via DRAM bounce
        nc.sync.dma_start(nxdram.ap(), nxrow[:])
        negpos = const.tile([128, 3 * SC], f32)
        nx_src = bass.AP(nxdram, 0, [[3, 128], [384, SC], [1, 3]])
        nc.sync.dma_start(
            negpos[:, :].rearrange("p (t j) -> p t j", j=3), nx_src)
        nc.scalar.activation(negc[:], negpos[:], Act.Copy, scale=-1.0)

        # ---- phase C: ball query + selection --------------------------
        with tc.tile_pool(name="bq", bufs=1) as bq, \
             tc.tile_pool(name="wps", bufs=1, space="PSUM") as wps:
            wrow_ps = [wps.tile([1, 512], f32, tag=f"wrow{fb}", name=f"wrow{fb}")
                       for fb in range(NFB)]
            iota1b = bq.tile([128, n], i16, tag="iota1b")
            nc.gpsimd.iota(iota1b[:], pattern=[[1, n]], base=1,
                           channel_multiplier=0)
            # xTrep[j][p, i] = xyz[i, j] replicated across partitions
            xTrep = []
            for j in range(3):
                t_ = bq.tile([128, n], f32, tag=f"xTrep{j}",
                             name=f"xTrep{j}")
                nc.sync.dma_start(t_[:],
                                  bass.AP(xyzT_d, j * n, [[0, 128], [1, n]]))
                xTrep.append(t_)
            for t in range(SC):
                sqs = []
                for j in range(3):
                    sq_ = bq.tile([128, n], f32, tag=f"bsq{j}")
                    nc.scalar.activation(sq_[:], xTrep[j][:], Act.Square,
                                         bias=negc[:, 3 * t + j:3 * t + j + 1])
                    sqs.append(sq_)
                nc.vector.tensor_tensor(sqs[0][:], sqs[0][:], sqs[1][:],
                                        Alu.add)
                nc.vector.tensor_tensor(sqs[0][:], sqs[0][:], sqs[2][:],
                                        Alu.add)
                hit = bq.tile([128, n], bf16, tag="hit")
                nc.vector.tensor_scalar(hit[:], sqs[0][:], RADIUS2, None,
                                        op0=Alu.is_le)
                rank = bq.tile([128, n], bf16, tag="rank")
                nc.vector.tensor_tensor_scan(
                    rank[:], hit[:], zeros1_bf[:, 0:1].broadcast_to([128, n]),
                    initial=0.0, op0=Alu.add, op1=Alu.add)
                nh = bq.tile([128, 1], bf16, tag="nh")
                nc.vector.tensor_copy(nh[:], rank[:, n - 1:n])
                m32 = bq.tile([128, n], bf16, tag="m32")
                nc.vector.tensor_scalar(m32[:], rank[:], float(ns), None,
                                        op0=Alu.is_le)
                mm = bq.tile([128, n], bf16, tag="mm")       # selmask
                nc.vector.tensor_tensor(mm[:], m32[:], hit[:], Alu.mult)
                nc.vector.tensor_tensor(rank[:], rank[:], mm[:], Alu.mult)
                slot = bq.tile([128, n], i16, tag="slot")
                nc.vector.tensor_scalar(slot[:], rank[:], -1.0, None,
                                        op0=Alu.add)
                # scatter point index (1-based) into its slot
                dsts = []
                for sub in range(NSCATTER):
                    dst = bq.tile([128, 34], i16, tag=f"scat{sub}")
                    nc.gpsimd.local_scatter(
                        dst[:], iota1b[:, sub * NSUB:(sub + 1) * NSUB],
                        slot[:, sub * NSUB:(sub + 1) * NSUB],
                        channels=128, num_elems=34, num_idxs=NSUB)
                    dsts.append(dst)
                merged = dsts[0]
                for mi in range(1, len(dsts)):
                    mg = bq.tile([128, 34], i16, tag=f"mg{mi}", name=f"mg{mi}")
                    nc.vector.tensor_tensor(mg[:], merged[:], dsts[mi][:],
                                            Alu.max)
                    merged = mg
                iszero = bq.tile([128, 34], i16, tag="iszero")
                nc.vector.tensor_scalar(iszero[:], merged[:], 0.0, None,
                                        op0=Alu.is_equal)
                padded = bq.tile([128, 34], i16, tag="padded")
                nc.vector.select(padded[:], iszero[:],
                                 merged[:, 0:1].broadcast_to([128, 34]),
                                 merged[:])
                gidx0 = bq.tile([128, 34], i16, tag="gidx0")
                nc.vector.tensor_scalar(gidx0[:], padded[:], -1.0, None,
                                        op0=Alu.add)
                nc.sync.dma_start(gidxdram.ap()[t * 128:(t + 1) * 128, :],
                                  gidx0[:, 0:ns])
                # multiplicity column sums (real hits only)
                for fb in range(NFB):
                    nc.tensor.matmul(wrow_ps[fb][:],
                                     lhsT=ones128bf[:],
                                     rhs=mm[:, fb * 512:(fb + 1) * 512],
                                     start=(t == 0), stop=(t == SC - 1),
                                     skip_group_check=True)
                # padding count: 32 - min(nhits, 32)
                p1 = bq.tile([128, 1], f32, tag="p1")
                nc.vector.tensor_scalar(p1[:], nh[:], float(ns), -1.0,
                                        op0=Alu.min, op1=Alu.mult)
                nc.vector.tensor_scalar(padcnt_all[:, t:t + 1], p1[:],
                                        float(ns), None, op0=Alu.add)
            # w row -> DRAM -> wT [128, PC]
            wrow_sb = const.tile([1, n], f32)
            for fb in range(NFB):
                nc.scalar.copy(wrow_sb[:, fb * 512:(fb + 1) * 512],
                               wrow_ps[fb][:])
            nc.sync.dma_start(wdram.ap(), wrow_sb[:])
        wt_src = bass.AP(wdram, 0, [[1, 128], [128, PC]])
        nc.sync.dma_start(wT[:], wt_src)

        if stop_after == "C":
            dbgc = const.tile([128, ns], f32)
            gsrc = bass.AP(gidxdram, 0, [[ns, 128], [1, ns]])
            dbg16 = const.tile([128, ns], i16)
            nc.sync.dma_start(dbg16[:], gsrc)
            nc.vector.tensor_copy(dbgc[:], dbg16[:])
            nc.sync.dma_start(out_d.ap()[0:128, 0:ns], dbgc[:])
            nc.sync.dma_start(out_d.ap()[0:128, ns:ns + SC], padcnt_all[:])
            nc.sync.dma_start(out_d.ap()[0:128, ns + SC:ns + SC + PC], wT[:])
            raise _StopEmit()

        # ---- phase D: gather + max-pool + padding stats terms ---------
        with tc.tile_pool(name="gat", bufs=2) as gat, \
             tc.tile_pool(name="sps", bufs=1, space="PSUM") as sps:
            ssum_ps = sps.tile([1, cout], f32, tag="ssum")
            ssq_ps = sps.tile([1, cout], f32, tag="ssq")
            for t in range(SC):
                idxs_t = gat.tile([128, 8 * ns], i16, tag="idxs")
                for g in range(8):
                    src = bass.AP(gidxdram, t * 128 * ns,
                                  [[ns, 16], [1, ns], [16 * ns, 8]])
                    nc.sync.dma_start(
                        idxs_t[16 * g:16 * (g + 1), :]
                        .rearrange("p (j g) -> p j g", g=8), src)
                if stop_after == "D1":
                    dbgi = gat.tile([128, 8 * ns], f32, tag="dbgi")
                    nc.vector.tensor_copy(dbgi[:], idxs_t[:])
                    nc.sync.dma_start(out_d.ap()[0:128, 0:8 * ns], dbgi[:])
                    raise _StopEmit()
                G = gat.tile([128, ns * cout], f32, tag="G")
                NI = 1024          # dma_gather fails above 1024 indices/call
                for gg in range(128 * ns // NI):
                    jpg = NI // 128
                    nc.gpsimd.dma_gather(
                        G[:, :].rearrange("p (j c) -> p j c", j=ns)
                        [:, gg * jpg:(gg + 1) * jpg, :],
                        qdram.ap(),
                        idxs_t[:, gg * (NI // 16):(gg + 1) * (NI // 16)],
                        num_idxs=NI, num_idxs_reg=NI, elem_size=cout)
                if stop_after == "D2":
                    nc.sync.dma_start(out_d.ap()[0:128, 0:cout],
                                      G[:, 0:cout])
                    raise _StopEmit()
                nc.vector.tensor_reduce(
                    pooled_all[:, t * cout:(t + 1) * cout],
                    G[:, :].rearrange("p (j c) -> p c j", j=ns),
                    axis=mybir.AxisListType.X, op=Alu.max)
                if stop_after != "D":
                    g0 = G[:, :].rearrange("p (j c) -> p j c", j=ns)[:, 0, :]
                    g0sq = gat.tile([128, cout], f32, tag="g0sq")
                    nc.scalar.activation(g0sq[:], g0, Act.Square)
                    nc.tensor.matmul(ssum_ps[:], lhsT=padcnt_all[:, t:t + 1],
                                     rhs=g0, start=(t == 0), stop=False,
                                     skip_group_check=True)
                    nc.tensor.matmul(ssq_ps[:], lhsT=padcnt_all[:, t:t + 1],
                                     rhs=g0sq[:], start=(t == 0), stop=False,
                                     skip_group_check=True)

            if stop_after == "D":
                nc.sync.dma_start(out_d.ap()[0:128, :],
                                  pooled_all[:, 0:cout])
                raise _StopEmit()

            # ---- phase E: weighted sums + AllReduce + BN apply --------
            for c in range(PC):
                qr = gat.tile([128, cout], f32, tag="qr")
                nc.sync.dma_start(qr[:], qdram.ap()[c * 128:(c + 1) * 128, :])
                qs = gat.tile([128, cout], f32, tag="qs")
                nc.scalar.activation(qs[:], qr[:], Act.Square)
                nc.tensor.matmul(ssum_ps[:], lhsT=wT[:, c:c + 1], rhs=qr[:],
                                 start=False, stop=(c == PC - 1),
                                 skip_group_check=True)
                nc.tensor.matmul(ssq_ps[:], lhsT=wT[:, c:c + 1], rhs=qs[:],
                                 start=False, stop=(c == PC - 1),
                                 skip_group_check=True)
            ccin_sb = gat.tile([1, 2 * cout], f32, tag="ccin")
            nc.scalar.copy(ccin_sb[:, 0:cout], ssum_ps[:])
            nc.scalar.copy(ccin_sb[:, cout:2 * cout], ssq_ps[:])
            nc.gpsimd.dma_start(ccin_d.ap(), ccin_sb[:])
            nc.gpsimd.collective_compute(
                "AllReduce", mybir.AluOpType.add,
                replica_groups=[list(range(n_cores))],
                ins=[ccin_d.ap().opt()], outs=[ccout_d.ap().opt()])
            ccsum = gat.tile([1, 2 * cout], f32, tag="ccsum")
            nc.gpsimd.dma_start(ccsum[:], ccout_d.ap())

            meanr = gat.tile([1, cout], f32, tag="meanr")
            nc.vector.tensor_scalar(meanr[:], ccsum[:, 0:cout], 1.0 / CNT,
                                    None, op0=Alu.mult)
            ex2 = gat.tile([1, cout], f32, tag="ex2")
            nc.vector.tensor_scalar(ex2[:], ccsum[:, cout:2 * cout], 1.0 / CNT,
                                    None, op0=Alu.mult)
            msq = gat.tile([1, cout], f32, tag="msq")
            nc.vector.tensor_tensor(msq[:], meanr[:], meanr[:], Alu.mult)
            var = gat.tile([1, cout], f32, tag="var")
            nc.vector.tensor_tensor(var[:], ex2[:], msq[:], Alu.subtract)
            varp = gat.tile([1, cout], f32, tag="varp")
            nc.vector.tensor_scalar(varp[:], var[:], BN_EPS, None, op0=Alu.add)
            sd = gat.tile([1, cout], f32, tag="sd")
            nc.scalar.activation(sd[:], varp[:], Act.Sqrt)
            inv = gat.tile([1, cout], f32, tag="inv")
            nc.vector.reciprocal(inv[:], sd[:])
            Arow = gat.tile([1, cout], f32, tag="Arow")
            nc.vector.tensor_tensor(Arow[:], inv[:], gammarow[:], Alu.mult)
            mA = gat.tile([1, cout], f32, tag="mA")
            nc.vector.tensor_tensor(mA[:], meanr[:], Arow[:], Alu.mult)
            Bbrow = gat.tile([1, cout], f32, tag="Bbrow")
            nc.vector.tensor_tensor(Bbrow[:], betarow[:], mA[:], Alu.subtract)

            Aps = sps.tile([128, cout], f32, tag="Aps")
            nc.tensor.matmul(Aps[:], lhsT=onesK1[:], rhs=Arow[:],
                             start=True, stop=True)
            Arep = gat.tile([128, cout], f32, tag="Arep")
            nc.scalar.copy(Arep[:], Aps[:])
            Bps = sps.tile([128, cout], f32, tag="Bps")
            nc.tensor.matmul(Bps[:], lhsT=onesK1[:], rhs=Bbrow[:],
                             start=True, stop=True)
            Brep = gat.tile([128, cout], f32, tag="Brep")
            nc.scalar.copy(Brep[:], Bps[:])

            for t in range(SC):
                x1 = gat.tile([128, cout], f32, tag="x1")
                nc.vector.tensor_tensor(
                    x1[:], pooled_all[:, t * cout:(t + 1) * cout], Arep[:],
                    Alu.mult)
                x2 = gat.tile([128, cout], f32, tag="x2")
                nc.vector.tensor_tensor(x2[:], x1[:], Brep[:], Alu.add)
                x3 = gat.tile([128, cout], f32, tag="x3")
                nc.scalar.activation(x3[:], x2[:], Act.Relu)
                nc.sync.dma_start(out_d.ap()[t * 128:(t + 1) * 128, :], x3[:])

      except _StopEmit:
        ctx.close()
    nc.compile()
    return nc


def make_in_maps(xyz, points, W1, b1, Wc, bc, gamma, beta):
    """Per-core input dicts (core i <- batch element i)."""
    bsz = xyz.shape[0]
    f32 = np.float32
    maps = []
    for i in range(bsz):
        maps.append({
            "xyzT": np.ascontiguousarray(xyz[i].T, dtype=f32),
            "pointsT": np.ascontiguousarray(points[i].T, dtype=f32),
            "W1": np.ascontiguousarray(W1, dtype=f32),
            "b1": np.ascontiguousarray(b1, dtype=f32).reshape(1, -1),
            "Wc": np.ascontiguousarray(Wc, dtype=f32),
            "bc": np.ascontiguousarray(bc, dtype=f32).reshape(1, -1),
            "gamma": np.ascontiguousarray(gamma, dtype=f32).reshape(1, -1),
            "beta": np.ascontiguousarray(beta, dtype=f32).reshape(1, -1),
        })
    return maps


_NC_CACHE = {}


def make_runner(nc, n_cores):
    """Build a reusable sharded-jit runner for `nc` (one compile per process).

    Mirrors concourse.bass2jax.run_bass_via_pjrt's multi-core path, but keeps
    the jitted callable so repeated invocations don't re-trace/re-compile.
    """
    import jax
    from jax.sharding import Mesh, PartitionSpec

    try:
        from jax.experimental.shard_map import shard_map
    except ImportError:  # newer jax
        from jax.sharding import shard_map
    from concourse import bass2jax, mybir

    bass2jax.install_neuronx_cc_hook()

    partition_name = (nc.partition_id_tensor.name
                      if nc.partition_id_tensor else None)
    in_names, out_names, out_avals, zero_outs = [], [], [], []
    for alloc in nc.m.functions[0].allocations:
        if not isinstance(alloc, mybir.MemoryLocationSet):
            continue
        name = alloc.memorylocations[0].name
        if alloc.kind == "ExternalInput":
            if name != partition_name:
                in_names.append(name)
        elif alloc.kind == "ExternalOutput":
            shape = tuple(alloc.tensor_shape)
            dtype = mybir.dt.np(alloc.dtype)
            out_names.append(name)
            out_avals.append(jax.core.ShapedArray(shape, dtype))
            zero_outs.append(np.zeros(shape, dtype))
    n_params = len(in_names)
    all_in_names = in_names + out_names
    if partition_name is not None:
        all_in_names = all_in_names + [partition_name]

    def _body(*args):
        operands = list(args)
        if partition_name is not None:
            operands.append(bass2jax.partition_id_tensor())
        outs = bass2jax._bass_exec_p.bind(
            *operands,
            out_avals=tuple(out_avals),
            in_names=tuple(all_in_names),
            out_names=tuple(out_names),
            lowering_input_output_aliases=(),
            sim_require_finite=True,
            sim_require_nnan=True,
            nc=nc,
        )
        return tuple(outs)

    devices = jax.devices()[:n_cores]
    mesh = Mesh(np.asarray(devices), ("core",))
    n_outs = len(out_names)
    sharded = jax.jit(
        shard_map(_body, mesh=mesh,
                  in_specs=(PartitionSpec("core"),) * (n_params + n_outs),
                  out_specs=(PartitionSpec("core"),) * n_outs,
                  check_rep=False),
        donate_argnums=tuple(range(n_params, n_params + n_outs)),
        keep_unused=True,
    )
    sharded_nodonate = jax.jit(
        shard_map(_body, mesh=mesh,
                  in_specs=(PartitionSpec("core"),) * (n_params + n_outs),
                  out_specs=(PartitionSpec("core"),) * n_outs,
                  check_rep=False),
        keep_unused=True,
    )

    def concat_inputs(in_maps):
        return [
            np.concatenate([np.asarray(in_maps[c][nm]) for c in range(n_cores)],
                           axis=0)
            for nm in in_names
        ]

    def fresh_zeros():
        return [np.zeros((n_cores * z.shape[0], *z.shape[1:]), z.dtype)
                for z in zero_outs]

    def run(in_maps):
        out_arrs = sharded(*concat_inputs(in_maps), *fresh_zeros())
        return [
            {nm: np.asarray(out_arrs[i]).reshape(n_cores, *out_avals[i].shape)[c]
             for i, nm in enumerate(out_names)}
            for c in range(n_cores)
        ]

    run.sharded = sharded
    run.sharded_nodonate = sharded_nodonate
    run.concat_inputs = concat_inputs
    run.fresh_zeros = fresh_zeros
    run.out_names = out_names
    run.out_avals = out_avals
    return run


def get_runner():
    if "runner" not in _NC_CACHE:
        nc = build_nc()
        _NC_CACHE["runner"] = make_runner(nc, B)
    return _NC_CACHE["runner"]


def kernel(xyz, t, points, W1, b1, Wc, bc, gamma, beta):
    del t  # time embedding is unused by the reference forward pass
    run = get_runner()
    in_maps = make_in_maps(xyz, points, W1, b1, Wc, bc, gamma, beta)
    res = run(in_maps)
    return np.stack([r["out"] for r in res]).astype(np.float32)



# revision 20
# speedup vs baseline: 12.5260x; 12.5260x over previous
"""Trainium2 Bass kernel for a PointNet++-style set-abstraction layer.

Per batch element: farthest-point sampling (1024 sequential steps), radius
ball-query grouping, pointwise MLP, 1x1 conv + global BatchNorm + ReLU,
neighborhood max-pool.  Data-parallel over batch: one batch element per
NeuronCore, with a single AllReduce for the BatchNorm statistics.

Key restructurings vs. the reference (validated to rel-err ~1e-6 on CPU):
  - FPS extracts the new centroid's coordinates via equality masks and
    cross-partition reductions (no integer argmax index needed).  The
    cross-partition max/sum broadcasts use GPSIMD partition_all_reduce
    (fps_impl="gpsimd") or PE transpose+matmul (fps_impl="pe").
  - Ball-query "first 32 in-radius indices" built with a prefix-scan rank +
    GPSIMD local_scatter (slot = rank-1 for hits with rank<=32).
  - q = (points @ W1 + b1) @ Wc + bc is computed once per point (instead of
    per gathered duplicate); the neighborhood max-pool is a DMA row gather of
    q followed by a free-axis max reduce.  max-pool commutes with the
    monotone BN+ReLU (gamma > 0), so BN is applied after pooling.
  - BN mean/var come from multiplicity-weighted sums: sum_i w_i q_i (+ a
    padding-duplicate correction term), AllReduced across cores.

build_nc(bodies=k) emits the whole computation k times back-to-back (with
per-body DRAM scratch) so test.py can measure per-body HW time as a slope,
amortizing the fixed per-dispatch axon-tunnel latency.
"""

import os
import sys

if "/opt/trn_rl_repo" not in sys.path:
    sys.path.insert(0, "/opt/trn_rl_repo")

import numpy as np

B = 8
N = 4096
S = 1024
NS = 32
CIN = 64
CMLP = 128
COUT = 256
RADIUS2 = float(np.float32(np.float64(0.15) * np.float64(0.15)))
BN_EPS = 1e-5


def build_nc(n=N, s=S, ns=NS, cin=CIN, cmlp=CMLP, cout=COUT, n_cores=B,
             batch_total=None, stop_after=None, fps_impl="gpsimd", bodies=1,
             gather_bf16=True):
    """Emit the Bass module (identical program on every core)."""
    from contextlib import ExitStack

    import concourse.bass as bass
    import concourse.tile as tile
    from concourse import bacc, bass_isa, mybir

    f32 = mybir.dt.float32
    bf16 = mybir.dt.bfloat16
    i16 = mybir.dt.int16
    Alu = mybir.AluOpType
    Act = mybir.ActivationFunctionType
    Red = bass_isa.ReduceOp

    FF = n // 128          # free elems per coordinate plane in FPS layout
    SC = s // 128          # center chunks
    PC = n // 128          # point chunks (q rows)
    NFB = n // 512         # 512-wide free blocks of n
    NSCATTER = 4           # local_scatter sub-calls per center chunk
    NSUB = n // NSCATTER
    if batch_total is None:
        batch_total = n_cores
    CNT = float(batch_total * s * ns)

    class _StopEmit(Exception):
        pass

    nc = bacc.Bacc("TRN2", target_bir_lowering=False, debug=False,
                   num_devices=n_cores)

    xyzT_d = nc.dram_tensor("xyzT", [3, n], f32, kind="ExternalInput")
    pointsT_d = nc.dram_tensor("pointsT", [cin, n], f32, kind="ExternalInput")
    W1_d = nc.dram_tensor("W1", [cin, cmlp], f32, kind="ExternalInput")
    b1_d = nc.dram_tensor("b1", [1, cmlp], f32, kind="ExternalInput")
    Wc_d = nc.dram_tensor("Wc", [cmlp, cout], f32, kind="ExternalInput")
    bc_d = nc.dram_tensor("bc", [1, cout], f32, kind="ExternalInput")
    gamma_d = nc.dram_tensor("gamma", [1, cout], f32, kind="ExternalInput")
    beta_d = nc.dram_tensor("beta", [1, cout], f32, kind="ExternalInput")
    out_d = nc.dram_tensor("out", [s, cout], f32, kind="ExternalOutput")

    with tile.TileContext(nc) as tc, ExitStack() as ctx:
      try:
        const = ctx.enter_context(tc.tile_pool(name="const", bufs=1))

        # ---- constant / input loads (shared across bodies) -------------
        W1_sb = const.tile([cin, cmlp], f32)
        nc.sync.dma_start(W1_sb[:], W1_d.ap())
        Wc_sb = const.tile([cmlp, cout], f32)
        nc.sync.dma_start(Wc_sb[:], Wc_d.ap())
        b1row = const.tile([1, cmlp], f32)
        nc.sync.dma_start(b1row[:], b1_d.ap())
        bcrow = const.tile([1, cout], f32)
        nc.sync.dma_start(bcrow[:], bc_d.ap())
        gammarow = const.tile([1, cout], f32)
        nc.sync.dma_start(gammarow[:], gamma_d.ap())
        betarow = const.tile([1, cout], f32)
        nc.sync.dma_start(betarow[:], beta_d.ap())
        ones512 = const.tile([1, 512], f32)
        nc.vector.memset(ones512[:], 1.0)
        onesK1 = const.tile([1, 128], f32)
        nc.vector.memset(onesK1[:], 1.0)
        ones128bf = const.tile([128, 1], bf16)
        nc.vector.memset(ones128bf[:], 1.0)
        zeros1_bf = const.tile([128, 1], bf16)
        nc.vector.memset(zeros1_bf[:], 0.0)
        # X3[p, j*FF + f] = xyz[p*FF + f, j]
        X3 = const.tile([128, 3 * FF], f32)
        for j in range(3):
            src = bass.AP(xyzT_d, j * n, [[FF, 128], [1, FF]])
            nc.sync.dma_start(X3[:, j * FF:(j + 1) * FF], src)
        X3v = X3[:, :].rearrange("p (j f) -> p j f", j=3)

        from concourse.masks import make_identity

        if fps_impl == "pe":
            ident = const.tile([128, 128], f32)
            make_identity(nc, ident[:])
            onesSQ = const.tile([128, 128], f32)
            nc.vector.memset(onesSQ[:], 1.0)

        def emit_body(bi, body):
            sfx = f"_b{bi}"
            qdram = nc.dram_tensor("qdram" + sfx, [n, cout], f32)
            gdt = bf16 if gather_bf16 else f32
            qdram_g = (nc.dram_tensor("qdramg" + sfx, [n, cout], bf16)
                       if gather_bf16 else qdram)
            gidxdram = nc.dram_tensor("gidxdram" + sfx, [s, ns], i16)
            wdram = nc.dram_tensor("wdram" + sfx, [1, n], f32)
            nxdram = nc.dram_tensor("nxdram" + sfx, [1, 3 * s], f32)
            ccin_d = nc.dram_tensor("ccin" + sfx, [1, 2 * cout], f32)
            ccout_d = nc.dram_tensor("ccout" + sfx, [1, 2 * cout], f32)

            negc = body.tile([128, 3 * SC], f32)     # -new_xyz, per-chunk cols
            nxrow = body.tile([1, 3 * s], f32)       # new_xyz as partition-0 row
            pooled_all = body.tile([128, SC * cout], f32)
            padcnt_all = body.tile([128, SC], f32)
            wT = body.tile([128, PC], f32)

            # ---- phase A: featsT = W1^T @ pointsT + b1; q rows -> qdram --
            with tc.tile_pool(name="psA" + sfx, bufs=2, space="PSUM") as psA, \
                 tc.tile_pool(name="qtmp" + sfx, bufs=3) as qtmp, \
                 tc.tile_pool(name="phA" + sfx, bufs=1) as phA:
                pointsT_sb = phA.tile([cin, n], f32)
                nc.sync.dma_start(pointsT_sb[:], pointsT_d.ap())
                featsT_sb = phA.tile([cmlp, n], f32)
                for blk in range(NFB):
                    ps = psA.tile([128, 512], f32, tag="fts")
                    nc.tensor.matmul(ps[:], lhsT=b1row[:], rhs=ones512[:],
                                     start=True, stop=False)
                    nc.tensor.matmul(ps[:], lhsT=W1_sb[:],
                                     rhs=pointsT_sb[:, blk * 512:(blk + 1) * 512],
                                     start=False, stop=True)
                    nc.scalar.copy(featsT_sb[:, blk * 512:(blk + 1) * 512],
                                   ps[:])
                for c in range(PC):
                    qp = psA.tile([128, cout], f32, tag="qp")
                    nc.tensor.matmul(qp[:], lhsT=onesK1[:], rhs=bcrow[:],
                                     start=True, stop=False)
                    nc.tensor.matmul(qp[:],
                                     lhsT=featsT_sb[:, c * 128:(c + 1) * 128],
                                     rhs=Wc_sb[:], start=False, stop=True)
                    qrow = qtmp.tile([128, cout], f32, tag="qrow")
                    nc.scalar.copy(qrow[:], qp[:])
                    nc.sync.dma_start(qdram.ap()[c * 128:(c + 1) * 128, :],
                                      qrow[:])
                    if gather_bf16:
                        qrow_g = qtmp.tile([128, cout], bf16, tag="qrowg")
                        nc.vector.tensor_copy(qrow_g[:], qp[:])
                        nc.sync.dma_start(
                            qdram_g.ap()[c * 128:(c + 1) * 128, :], qrow_g[:])

            if stop_after == "A":
                dbg = body.tile([128, cout], f32)
                nc.sync.dma_start(dbg[:], qdram.ap()[0:128, :])
                nc.sync.dma_start(out_d.ap()[0:128, :], dbg[:])
                raise _StopEmit()

            # ---- phase B: farthest point sampling -----------------------
            # Cross-partition argmax/extraction: equality mask + reduction.
            with tc.tile_pool(name="fps" + sfx, bufs=2) as fp, \
                 tc.tile_pool(name="fpp" + sfx, bufs=2, space="PSUM") as fpp, \
                 tc.tile_pool(name="fps1" + sfx, bufs=1) as fp1:
                dmin = fp1.tile([128, FF], f32)
                nc.vector.memset(dmin[:], 1e10)

                def extract_tail(mx3, k):
                    """masked point's coords summed over all points ->
                    csumB [128,3] (equal on every partition); coords also
                    appended to nxrow (partition-0 row)."""
                    m3 = fp.tile([128, 3], f32, tag="m3")
                    nc.vector.tensor_reduce(
                        m3[:], mx3[:, :].rearrange("p (j f) -> p j f", j=3),
                        axis=mybir.AxisListType.X, op=Alu.add)
                    if fps_impl == "gpsimd":
                        csumB = fp.tile([128, 3], f32, tag="csumB")
                        nc.gpsimd.partition_all_reduce(csumB[:], m3[:],
                                                       channels=128,
                                                       reduce_op=Red.add)
                    else:
                        csumB = fpp.tile([128, 3], f32, tag="csumB")
                        nc.tensor.matmul(csumB[:], lhsT=onesSQ[:], rhs=m3[:],
                                         start=True, stop=True)
                    nc.scalar.copy(nxrow[:, 3 * k:3 * k + 3], csumB[0:1, :])
                    return csumB

                mx0 = fp1.tile([128, 3 * FF], f32)
                nc.vector.memset(mx0[:], 0.0)
                for j in range(3):
                    nc.vector.tensor_copy(mx0[0:1, j * FF:j * FF + 1],
                                          X3[0:1, j * FF:j * FF + 1])
                csumB = extract_tail(mx0, 0)

                for k in range(1, s):
                    diff = fp.tile([128, 3 * FF], f32, tag="diff")
                    nc.vector.tensor_tensor(
                        diff[:, :].rearrange("p (j f) -> p j f", j=3), X3v,
                        csumB[:, :].unsqueeze(2).broadcast_to([128, 3, FF]),
                        Alu.subtract)
                    sq = fp.tile([128, 3 * FF], f32, tag="sq")
                    nc.vector.tensor_tensor(sq[:], diff[:], diff[:], Alu.mult)
                    d = fp.tile([128, FF], f32, tag="d")
                    nc.vector.tensor_reduce(
                        d[:], sq[:, :].rearrange("p (j f) -> p f j", j=3),
                        axis=mybir.AxisListType.X, op=Alu.add)
                    dmin2 = fp.tile([128, FF], f32, tag="dmin2")
                    nc.vector.tensor_tensor(dmin2[:], d[:], dmin[:], Alu.min)
                    dmin = dmin2
                    permax = fp.tile([128, 1], f32, tag="permax")
                    nc.vector.tensor_reduce(permax[:], dmin2[:],
                                            axis=mybir.AxisListType.X,
                                            op=Alu.max)
                    if fps_impl == "gpsimd":
                        gmaxB = fp.tile([128, 1], f32, tag="gmaxB")
                        nc.gpsimd.partition_all_reduce(gmaxB[:], permax[:],
                                                       channels=128,
                                                       reduce_op=Red.max)
                    else:
                        pmT = fpp.tile([1, 128], f32, tag="pmT")
                        nc.tensor.transpose(pmT[:], permax[:], ident[:])
                        gmax = fp.tile([1, 1], f32, tag="gmax")
                        nc.vector.tensor_reduce(gmax[:], pmT[:],
                                                axis=mybir.AxisListType.X,
                                                op=Alu.max)
                        gmaxB = fpp.tile([128, 1], f32, tag="gmaxB")
                        nc.tensor.matmul(gmaxB[:], lhsT=onesK1[:], rhs=gmax[:],
                                         start=True, stop=True)
                    # fused mask+select: mx3 = (dmin2 == gmax) * X3
                    mx3 = fp.tile([128, 3 * FF], f32, tag="mx3")
                    nc.vector.scalar_tensor_tensor(
                        mx3[:, :].rearrange("p (j f) -> p j f", j=3),
                        dmin2[:, :].unsqueeze(1).broadcast_to([128, 3, FF]),
                        gmaxB[:, 0:1], X3v, op0=Alu.is_equal, op1=Alu.mult)
                    csumB = extract_tail(mx3, k)

            if stop_after == "B":
                nc.sync.dma_start(out_d.ap()[0:1, 0:cout], nxrow[:, 0:cout])
                raise _StopEmit()

            # negc[p, 3t+j] = -new_xyz[128t+p, j] via DRAM bounce
            nc.sync.dma_start(nxdram.ap(), nxrow[:])
            negpos = body.tile([128, 3 * SC], f32)
            nx_src = bass.AP(nxdram, 0, [[3, 128], [384, SC], [1, 3]])
            nc.sync.dma_start(
                negpos[:, :].rearrange("p (t j) -> p t j", j=3), nx_src)
            nc.scalar.activation(negc[:], negpos[:], Act.Copy, scale=-1.0)

            # ---- phase C: ball query + selection ------------------------
            with tc.tile_pool(name="bq" + sfx, bufs=1) as bq, \
                 tc.tile_pool(name="bqs" + sfx, bufs=2) as bqs, \
                 tc.tile_pool(name="wps" + sfx, bufs=1, space="PSUM") as wps:
                wrow_ps = [wps.tile([1, 512], f32, tag=f"wrow{fb}",
                                    name=f"wrow{fb}")
                           for fb in range(NFB)]
                iota1b = bq.tile([128, n], i16, tag="iota1b")
                nc.gpsimd.iota(iota1b[:], pattern=[[1, n]], base=1,
                               channel_multiplier=0)
                # xTrep[j][p, i] = xyz[i, j] replicated across partitions
                xTrep = []
                for j in range(3):
                    t_ = bq.tile([128, n], f32, tag=f"xTrep{j}",
                                 name=f"xTrep{j}")
                    nc.sync.dma_start(
                        t_[:], bass.AP(xyzT_d, j * n, [[0, 128], [1, n]]))
                    xTrep.append(t_)
                for t in range(SC):
                    sqs = []
                    for j in range(3):
                        sq_ = bqs.tile([128, n], f32, tag=f"bsq{j}")
                        nc.scalar.activation(sq_[:], xTrep[j][:], Act.Square,
                                             bias=negc[:, 3 * t + j:3 * t + j + 1])
                        sqs.append(sq_)
                    nc.vector.tensor_tensor(sqs[0][:], sqs[0][:], sqs[1][:],
                                            Alu.add)
                    nc.vector.tensor_tensor(sqs[0][:], sqs[0][:], sqs[2][:],
                                            Alu.add)
                    hit = bq.tile([128, n], bf16, tag="hit")
                    nc.vector.tensor_scalar(hit[:], sqs[0][:], RADIUS2, None,
                                            op0=Alu.is_le)
                    rank = bq.tile([128, n], bf16, tag="rank")
                    nc.vector.tensor_tensor_scan(
                        rank[:], hit[:],
                        zeros1_bf[:, 0:1].broadcast_to([128, n]),
                        initial=0.0, op0=Alu.add, op1=Alu.add)
                    nh = bq.tile([128, 1], bf16, tag="nh")
                    nc.vector.tensor_copy(nh[:], rank[:, n - 1:n])
                    mm = bq.tile([128, n], bf16, tag="mm")       # selmask
                    nc.vector.scalar_tensor_tensor(
                        mm[:], rank[:], float(ns), hit[:], op0=Alu.is_le,
                        op1=Alu.mult)
                    nc.vector.tensor_tensor(rank[:], rank[:], mm[:], Alu.mult)
                    slot = bq.tile([128, n], i16, tag="slot")
                    nc.vector.tensor_scalar(slot[:], rank[:], -1.0, None,
                                            op0=Alu.add)
                    # scatter point index (1-based) into its slot
                    dsts = []
                    for sub in range(NSCATTER):
                        dst = bq.tile([128, 34], i16, tag=f"scat{sub}")
                        nc.gpsimd.local_scatter(
                            dst[:], iota1b[:, sub * NSUB:(sub + 1) * NSUB],
                            slot[:, sub * NSUB:(sub + 1) * NSUB],
                            channels=128, num_elems=34, num_idxs=NSUB)
                        dsts.append(dst)
                    merged = dsts[0]
                    for mi in range(1, len(dsts)):
                        mg = bq.tile([128, 34], i16, tag=f"mg{mi}",
                                     name=f"mg{mi}")
                        nc.vector.tensor_tensor(mg[:], merged[:], dsts[mi][:],
                                                Alu.max)
                        merged = mg
                    iszero = bq.tile([128, 34], i16, tag="iszero")
                    nc.vector.tensor_scalar(iszero[:], merged[:], 0.0, None,
                                            op0=Alu.is_equal)
                    padded = bq.tile([128, 34], i16, tag="padded")
                    nc.vector.select(padded[:], iszero[:],
                                     merged[:, 0:1].broadcast_to([128, 34]),
                                     merged[:])
                    gidx0 = bq.tile([128, 34], i16, tag="gidx0")
                    nc.vector.tensor_scalar(gidx0[:], padded[:], -1.0, None,
                                            op0=Alu.add)
                    nc.sync.dma_start(gidxdram.ap()[t * 128:(t + 1) * 128, :],
                                      gidx0[:, 0:ns])
                    # multiplicity column sums (real hits only)
                    for fb in range(NFB):
                        nc.tensor.matmul(wrow_ps[fb][:],
                                         lhsT=ones128bf[:],
                                         rhs=mm[:, fb * 512:(fb + 1) * 512],
                                         start=(t == 0), stop=(t == SC - 1),
                                         skip_group_check=True)
                    # padding count: 32 - min(nhits, 32)
                    p1 = bq.tile([128, 1], f32, tag="p1")
                    nc.vector.tensor_scalar(p1[:], nh[:], float(ns), -1.0,
                                            op0=Alu.min, op1=Alu.mult)
                    nc.vector.tensor_scalar(padcnt_all[:, t:t + 1], p1[:],
                                            float(ns), None, op0=Alu.add)
                # w row -> DRAM -> wT [128, PC]
                wrow_sb = body.tile([1, n], f32)
                for fb in range(NFB):
                    nc.scalar.copy(wrow_sb[:, fb * 512:(fb + 1) * 512],
                                   wrow_ps[fb][:])
                nc.sync.dma_start(wdram.ap(), wrow_sb[:])
            wt_src = bass.AP(wdram, 0, [[1, 128], [128, PC]])
            nc.sync.dma_start(wT[:], wt_src)

            if stop_after == "C":
                dbgc = body.tile([128, ns], f32)
                gsrc = bass.AP(gidxdram, 0, [[ns, 128], [1, ns]])
                dbg16 = body.tile([128, ns], i16)
                nc.sync.dma_start(dbg16[:], gsrc)
                nc.vector.tensor_copy(dbgc[:], dbg16[:])
                nc.sync.dma_start(out_d.ap()[0:128, 0:ns], dbgc[:])
                nc.sync.dma_start(out_d.ap()[0:128, ns:ns + SC],
                                  padcnt_all[:])
                nc.sync.dma_start(out_d.ap()[0:128, ns + SC:ns + SC + PC],
                                  wT[:])
                raise _StopEmit()

            # ---- phase D: gather + max-pool + padding stats terms -------
            with tc.tile_pool(name="gat" + sfx, bufs=2) as gat, \
                 tc.tile_pool(name="sps" + sfx, bufs=1, space="PSUM") as sps:
                ssum_ps = sps.tile([1, cout], f32, tag="ssum")
                ssq_ps = sps.tile([1, cout], f32, tag="ssq")
                for t in range(SC):
                    idxs_t = gat.tile([128, 8 * ns], i16, tag="idxs")
                    for g in range(8):
                        src = bass.AP(gidxdram, t * 128 * ns,
                                      [[ns, 16], [1, ns], [16 * ns, 8]])
                        nc.sync.dma_start(
                            idxs_t[16 * g:16 * (g + 1), :]
                            .rearrange("p (j g) -> p j g", g=8), src)
                    G = gat.tile([128, ns * cout], gdt, tag="G")
                    NI = 1024      # dma_gather fails above 1024 indices/call
                    for gg in range(128 * ns // NI):
                        jpg = NI // 128
                        nc.gpsimd.dma_gather(
                            G[:, :].rearrange("p (j c) -> p j c", j=ns)
                            [:, gg * jpg:(gg + 1) * jpg, :],
                            qdram_g.ap(),
                            idxs_t[:, gg * (NI // 16):(gg + 1) * (NI // 16)],
                            num_idxs=NI, num_idxs_reg=NI, elem_size=cout)
                    if gather_bf16:
                        pool_bf = gat.tile([128, cout], bf16, tag="poolbf")
                        nc.vector.tensor_reduce(
                            pool_bf[:],
                            G[:, :].rearrange("p (j c) -> p c j", j=ns),
                            axis=mybir.AxisListType.X, op=Alu.max)
                        nc.vector.tensor_copy(
                            pooled_all[:, t * cout:(t + 1) * cout], pool_bf[:])
                    else:
                        nc.vector.tensor_reduce(
                            pooled_all[:, t * cout:(t + 1) * cout],
                            G[:, :].rearrange("p (j c) -> p c j", j=ns),
                            axis=mybir.AxisListType.X, op=Alu.max)
                    g0v = G[:, :].rearrange("p (j c) -> p j c", j=ns)[:, 0, :]
                    if gather_bf16:
                        g0 = gat.tile([128, cout], f32, tag="g0f")
                        nc.vector.tensor_copy(g0[:], g0v)
                        g0 = g0[:]
                    else:
                        g0 = g0v
                    g0sq = gat.tile([128, cout], f32, tag="g0sq")
                    nc.scalar.activation(g0sq[:], g0, Act.Square)
                    nc.tensor.matmul(ssum_ps[:], lhsT=padcnt_all[:, t:t + 1],
                                     rhs=g0, start=(t == 0), stop=False,
                                     skip_group_check=True)
                    nc.tensor.matmul(ssq_ps[:], lhsT=padcnt_all[:, t:t + 1],
                                     rhs=g0sq[:], start=(t == 0), stop=False,
                                     skip_group_check=True)

                if stop_after == "D":
                    nc.sync.dma_start(out_d.ap()[0:128, :],
                                      pooled_all[:, 0:cout])
                    raise _StopEmit()

                # ---- phase E: weighted sums + AllReduce + BN apply ------
                EB = 4          # q-row chunks per DMA
                for cb in range(PC // EB):
                    qr = gat.tile([128, EB * cout], f32, tag="qr")
                    src = bass.AP(qdram, cb * EB * 128 * cout,
                                  [[cout, 128], [128 * cout, EB], [1, cout]])
                    nc.sync.dma_start(
                        qr[:, :].rearrange("p (g c) -> p g c", g=EB), src)
                    qs = gat.tile([128, EB * cout], f32, tag="qs")
                    nc.scalar.activation(qs[:], qr[:], Act.Square)
                    for g in range(EB):
                        c = cb * EB + g
                        nc.tensor.matmul(ssum_ps[:], lhsT=wT[:, c:c + 1],
                                         rhs=qr[:, g * cout:(g + 1) * cout],
                                         start=False,
                                         stop=(c == PC - 1),
                                         skip_group_check=True)
                        nc.tensor.matmul(ssq_ps[:], lhsT=wT[:, c:c + 1],
                                         rhs=qs[:, g * cout:(g + 1) * cout],
                                         start=False,
                                         stop=(c == PC - 1),
                                         skip_group_check=True)
                ccin_sb = gat.tile([1, 2 * cout], f32, tag="ccin")
                nc.scalar.copy(ccin_sb[:, 0:cout], ssum_ps[:])
                nc.scalar.copy(ccin_sb[:, cout:2 * cout], ssq_ps[:])
                nc.gpsimd.dma_start(ccin_d.ap(), ccin_sb[:])
                nc.gpsimd.collective_compute(
                    "AllReduce", mybir.AluOpType.add,
                    replica_groups=[list(range(n_cores))],
                    ins=[ccin_d.ap().opt()], outs=[ccout_d.ap().opt()])
                ccsum = gat.tile([1, 2 * cout], f32, tag="ccsum")
                nc.gpsimd.dma_start(ccsum[:], ccout_d.ap())

                meanr = gat.tile([1, cout], f32, tag="meanr")
                nc.vector.tensor_scalar(meanr[:], ccsum[:, 0:cout], 1.0 / CNT,
                                        None, op0=Alu.mult)
                ex2 = gat.tile([1, cout], f32, tag="ex2")
                nc.vector.tensor_scalar(ex2[:], ccsum[:, cout:2 * cout],
                                        1.0 / CNT, None, op0=Alu.mult)
                msq = gat.tile([1, cout], f32, tag="msq")
                nc.vector.tensor_tensor(msq[:], meanr[:], meanr[:], Alu.mult)
                var = gat.tile([1, cout], f32, tag="var")
                nc.vector.tensor_tensor(var[:], ex2[:], msq[:], Alu.subtract)
                varp = gat.tile([1, cout], f32, tag="varp")
                nc.vector.tensor_scalar(varp[:], var[:], BN_EPS, None,
                                        op0=Alu.add)
                sd = gat.tile([1, cout], f32, tag="sd")
                nc.scalar.activation(sd[:], varp[:], Act.Sqrt)
                inv = gat.tile([1, cout], f32, tag="inv")
                nc.vector.reciprocal(inv[:], sd[:])
                Arow = gat.tile([1, cout], f32, tag="Arow")
                nc.vector.tensor_tensor(Arow[:], inv[:], gammarow[:],
                                        Alu.mult)
                mA = gat.tile([1, cout], f32, tag="mA")
                nc.vector.tensor_tensor(mA[:], meanr[:], Arow[:], Alu.mult)
                Bbrow = gat.tile([1, cout], f32, tag="Bbrow")
                nc.vector.tensor_tensor(Bbrow[:], betarow[:], mA[:],
                                        Alu.subtract)

                Aps = sps.tile([128, cout], f32, tag="Aps")
                nc.tensor.matmul(Aps[:], lhsT=onesK1[:], rhs=Arow[:],
                                 start=True, stop=True)
                Arep = gat.tile([128, cout], f32, tag="Arep")
                nc.scalar.copy(Arep[:], Aps[:])
                Bps = sps.tile([128, cout], f32, tag="Bps")
                nc.tensor.matmul(Bps[:], lhsT=onesK1[:], rhs=Bbrow[:],
                                 start=True, stop=True)
                Brep = gat.tile([128, cout], f32, tag="Brep")
                nc.scalar.copy(Brep[:], Bps[:])

                for t in range(SC):
                    x1 = gat.tile([128, cout], f32, tag="x1")
                    nc.vector.tensor_tensor(
                        x1[:], pooled_all[:, t * cout:(t + 1) * cout],
                        Arep[:], Alu.mult)
                    x2 = gat.tile([128, cout], f32, tag="x2")
                    nc.vector.tensor_tensor(x2[:], x1[:], Brep[:], Alu.add)
                    x3 = gat.tile([128, cout], f32, tag="x3")
                    nc.scalar.activation(x3[:], x2[:], Act.Relu)
                    nc.sync.dma_start(out_d.ap()[t * 128:(t + 1) * 128, :],
                                      x3[:])

        for bi in range(bodies):
            with tc.tile_pool(name=f"body_b{bi}", bufs=1) as body_pool:
                emit_body(bi, body_pool)

      except _StopEmit:
        ctx.close()
    nc.compile()
    return nc


def make_in_maps(xyz, points, W1, b1, Wc, bc, gamma, beta):
    """Per-core input dicts (core i <- batch element i)."""
    bsz = xyz.shape[0]
    f32 = np.float32
    maps = []
    for i in range(bsz):
        maps.append({
            "xyzT": np.ascontiguousarray(xyz[i].T, dtype=f32),
            "pointsT": np.ascontiguousarray(points[i].T, dtype=f32),
            "W1": np.ascontiguousarray(W1, dtype=f32),
            "b1": np.ascontiguousarray(b1, dtype=f32).reshape(1, -1),
            "Wc": np.ascontiguousarray(Wc, dtype=f32),
            "bc": np.ascontiguousarray(bc, dtype=f32).reshape(1, -1),
            "gamma": np.ascontiguousarray(gamma, dtype=f32).reshape(1, -1),
            "beta": np.ascontiguousarray(beta, dtype=f32).reshape(1, -1),
        })
    return maps


_NC_CACHE = {}


def make_runner(nc, n_cores):
    """Build a reusable sharded-jit runner for `nc` (one compile per process).

    Mirrors concourse.bass2jax.run_bass_via_pjrt's multi-core path, but keeps
    the jitted callable so repeated invocations don't re-trace/re-compile.
    """
    import jax
    from jax.sharding import Mesh, PartitionSpec

    try:
        from jax.experimental.shard_map import shard_map
    except ImportError:  # newer jax
        from jax.sharding import shard_map
    from concourse import bass2jax, mybir

    bass2jax.install_neuronx_cc_hook()

    partition_name = (nc.partition_id_tensor.name
                      if nc.partition_id_tensor else None)
    in_names, out_names, out_avals, zero_outs = [], [], [], []
    for alloc in nc.m.functions[0].allocations:
        if not isinstance(alloc, mybir.MemoryLocationSet):
            continue
        name = alloc.memorylocations[0].name
        if alloc.kind == "ExternalInput":
            if name != partition_name:
                in_names.append(name)
        elif alloc.kind == "ExternalOutput":
            shape = tuple(alloc.tensor_shape)
            dtype = mybir.dt.np(alloc.dtype)
            out_names.append(name)
            out_avals.append(jax.core.ShapedArray(shape, dtype))
            zero_outs.append(np.zeros(shape, dtype))
    n_params = len(in_names)
    all_in_names = in_names + out_names
    if partition_name is not None:
        all_in_names = all_in_names + [partition_name]

    def _body(*args):
        operands = list(args)
        if partition_name is not None:
            operands.append(bass2jax.partition_id_tensor())
        outs = bass2jax._bass_exec_p.bind(
            *operands,
            out_avals=tuple(out_avals),
            in_names=tuple(all_in_names),
            out_names=tuple(out_names),
            lowering_input_output_aliases=(),
            sim_require_finite=True,
            sim_require_nnan=True,
            nc=nc,
        )
        return tuple(outs)

    devices = jax.devices()[:n_cores]
    mesh = Mesh(np.asarray(devices), ("core",))
    n_outs = len(out_names)
    sharded = jax.jit(
        shard_map(_body, mesh=mesh,
                  in_specs=(PartitionSpec("core"),) * (n_params + n_outs),
                  out_specs=(PartitionSpec("core"),) * n_outs,
                  check_rep=False),
        donate_argnums=tuple(range(n_params, n_params + n_outs)),
        keep_unused=True,
    )
    sharded_nodonate = jax.jit(
        shard_map(_body, mesh=mesh,
                  in_specs=(PartitionSpec("core"),) * (n_params + n_outs),
                  out_specs=(PartitionSpec("core"),) * n_outs,
                  check_rep=False),
        keep_unused=True,
    )

    def concat_inputs(in_maps):
        return [
            np.concatenate([np.asarray(in_maps[c][nm]) for c in range(n_cores)],
                           axis=0)
            for nm in in_names
        ]

    def fresh_zeros():
        return [np.zeros((n_cores * z.shape[0], *z.shape[1:]), z.dtype)
                for z in zero_outs]

    def run(in_maps):
        out_arrs = sharded(*concat_inputs(in_maps), *fresh_zeros())
        return [
            {nm: np.asarray(out_arrs[i]).reshape(n_cores, *out_avals[i].shape)[c]
             for i, nm in enumerate(out_names)}
            for c in range(n_cores)
        ]

    run.sharded = sharded
    run.sharded_nodonate = sharded_nodonate
    run.concat_inputs = concat_inputs
    run.fresh_zeros = fresh_zeros
    run.out_names = out_names
    run.out_avals = out_avals
    return run


def get_runner():
    if "runner" not in _NC_CACHE:
        nc = build_nc()
        _NC_CACHE["runner"] = make_runner(nc, B)
    return _NC_CACHE["runner"]


def kernel(xyz, t, points, W1, b1, Wc, bc, gamma, beta):
    del t  # time embedding is unused by the reference forward pass
    run = get_runner()
    in_maps = make_in_maps(xyz, points, W1, b1, Wc, bc, gamma, beta)
    res = run(in_maps)
    return np.stack([r["out"] for r in res]).astype(np.float32)
